# revision 54
# baseline (speedup 1.0000x reference)
"""Trainium2 Bass kernel for nn_CausalMatchingModule (fp8 DoubleRow, v2).

Reference computation (B=64, N=16, D=512, P=64, L=8, D2=256):
  per modality feats in {img, text}:
    src = feats[:, paths[:, :-1]]          # [B,P,L,D]
    dst = feats[:, paths[:, 1:]]           # [B,P,L,D]
    h  = relu(concat(src,dst) @ W1 + b1)   # [B,P,L,D]
    h  = relu(h @ W2 + b2)                 # [B,P,L,D2]
    s  = sigmoid(h @ W3 + b3)[...,0]       # [B,P,L]
    pf = pad(s, L->D)                      # only first L rows of W4 matter
    a  = relu(pf @ W4 + b4)                # [B,P,D]
    y  = sigmoid(a @ W5 + b5)              # [B,P,1]
  out = max_p sqrt(y_img * y_text)         # [B,1]

Key algebraic restructuring (same as baseline):
  * concat(src,dst)@W1 factors through the gather: per-node projections
    F1/F2 on B*N rows; pair (i,j) pre-act = F1[b,i] + F2[b,j] + b1.
  * paths reference only N*N = 256 distinct pairs, so the edge MLP runs
    on the 256-pair table per batch; per-edge scores gathered afterwards
    with a one-hot matmul + strided pivot DMAs.
  * pf@W4 = s @ W4[:L,:];  max_p sqrt(m) = sqrt(max_p m).

v2 changes:
  * dummy Sigmoid first on ACT pins the sigmoid_and_others table set ->
    single ACT_TABLE_LOAD (no 1.3us reload before the z drain).
  * L3 / gather / final y matmuls DR-packed (half the instructions).
  * final y matmul in fp8 (W4/b4 scaled x16 so `at` drains to fp8,
    W5 x32 fp8); host divides by 512.
  * L2 loop order nn-outer so each nn's psums drain immediately.
  * input DMA triggers spread across engines, first-needed first.
  * lean TileContext teardown (single drain + gpsimd sem clears instead
    of two event-semaphore all-engine barrier rings) -- saves ~6us.
  * endgame reordered: gather(0) outprioritizes L2(1) on Tensor, stage2
    per modality emitted last.

Sharding: data-parallel over batch. Core c handles batches [8c, 8c+8);
weights replicated. Both modalities processed by one program.
"""

import os
import sys

import ml_dtypes
import numpy as np

BF16NP = ml_dtypes.bfloat16
F8NP = ml_dtypes.float8_e4m3

for _p in ("/opt/trn_rl_repo",):
    if os.path.isdir(_p) and _p not in sys.path:
        sys.path.append(_p)

import concourse.bacc as bacc
import concourse.bass as bass
import concourse.tile as tile
from concourse import mybir
from concourse.bass_utils import run_bass_kernel_spmd
from concourse.vector_clock import ScopedClock

F32 = mybir.dt.float32
BF16 = mybir.dt.bfloat16
F8 = mybir.dt.float8e4
AF = mybir.ActivationFunctionType
ALU = mybir.AluOpType
DR = mybir.MatmulPerfMode.DoubleRow

B, N, D, P, L = 64, 16, 512, 64, 8
D2 = D // 2
NCORES = 8
BC = B // NCORES          # 8 batches per core
R = BC * N                # 128 rows per modality
COLS = 2 * R              # 256 layer-1 rhs columns (both modalities)
NPAIR = N * N             # 256 (i,j) pairs per batch
HC_M = BC * NPAIR         # 2048 H columns per modality
WS1 = 32.0                # host scale on W1 (fp8 range)
WS2 = 4.0                 # host scale on W2: psum = 4*h2pre, drained scale-free
WS3 = 32.0                # host scale on W3
H2S = 4.0                 # h2 stored x4
AS = 16.0                 # host scale on W4/b4 -> `at` stored x16 in fp8
WS5 = 32.0                # host scale on W5 (fp8 range)
YS = AS * WS5             # y psum = 512 * ypre


def _lean_drain_and_barrier(self, tick_clock, wait_clock):
    """Teardown: single sync drain + gpsimd sem clears.

    Replaces the stock two event-semaphore all-engine barrier rings
    (~6us of serial event-queue latency).  Correctness: the sync drain
    waits for every semaphore's final value (all instructions + DMAs
    complete); gpsimd then clears the semaphores so the NEFF stays
    re-runnable; every other engine simply halts after its last real
    instruction.
    """
    nc = self.nc
    drain_inst = nc.sync.drain()
    wait_clock.add_sem_waits(drain_inst.ins, ScopedClock({None: tick_clock.global_clock}))
    sem = nc.alloc_semaphore("lean_teardown")
    drain_inst.then_inc(sem, 1)
    nc.gpsimd.wait_ge(sem, 1)
    popped = nc._tile_sem_poison_stack.pop()
    assert popped is self._sem_poison
    nc.clear_and_free_semaphores(list(self.sems.allocated().values()))
    nc.gpsimd.sem_clear(range(sem.num, sem.num + 1))


def _strided(base: bass.AP, dims) -> bass.AP:
    """AP with explicit free [step,count] dims (incl. stride-0 broadcast)."""
    return bass.AP(base.tensor, base.offset, [list(base.ap[0])] + [list(d) for d in dims])


def _build_program(b3: float, b5: float):
    nc = bacc.Bacc("TRN2", target_bir_lowering=False)

    # Consolidated inputs:
    #  xtp  [128, 1024] fp8: col = k*256 + modal*128 + i*8 + b   (X^T k-chunks)
    #  w1p  [128, 4096] fp8: col = ((m*2+h)*4+k)*128 + c         (32*W1)
    #  w2p  [128, 1092] fp8: [0:1024] 4*W2 (k*256+m2*128+c) | [1024:1088] 32*W3
    #                        rep x32 | [1088:1092] 32*W5 d-chunk cols
    #  wsb  [128, 512]  bf16: 16*W4' (only rows 0:L nonzero)
    #  gmat [128, 1024] fp8: one-hot gather, col = ec*512 + plc*128 + r
    #  bsf  [128, 12]   f32: [0:4] b1 | [4:6] 4*b2 | [6:10] 16*b4
    xtp = nc.dram_tensor("xtp", [128, 1024], F8, kind="ExternalInput")
    w1p = nc.dram_tensor("w1p", [128, 4096], F8, kind="ExternalInput")
    w2p = nc.dram_tensor("w2p", [128, 1120], F8, kind="ExternalInput")
    wsb = nc.dram_tensor("wsb", [128, 512], BF16, kind="ExternalInput")
    gmat = nc.dram_tensor("gmat", [128, 1024], F8, kind="ExternalInput")
    bsf = nc.dram_tensor("bsf", [128, 12], F32, kind="ExternalInput")
    out = nc.dram_tensor("out", [2, 512], F32, kind="ExternalOutput")

    with tile.TileContext(nc) as tc:
        from contextlib import ExitStack
        with (
            tc.tile_pool(name="wpool", bufs=1) as wpool,
            tc.tile_pool(name="fsb", bufs=2) as fsb_pool,
            tc.tile_pool(name="hbf", bufs=4) as hbf_pool,
            tc.tile_pool(name="hpool", bufs=1) as hpool,
            tc.tile_pool(name="h2pool", bufs=5) as h2pool,
            tc.tile_pool(name="stpool", bufs=1) as stpool,
            tc.tile_pool(name="ypool", bufs=1) as ypool,
        ):
            psum_stack = ExitStack()
            fpool = psum_stack.enter_context(tc.tile_pool(name="fps", bufs=1, space="PSUM"))
            l2ps_pool = psum_stack.enter_context(tc.tile_pool(name="l2ps", bufs=4, space="PSUM"))
            zps_pool = psum_stack.enter_context(tc.tile_pool(name="zps", bufs=1, space="PSUM"))
            # ---------------- input loads (first-needed first) ----------------
            xt_sb = wpool.tile([128, 1024], F8, tag="xt", name="xt")
            w1_sb = wpool.tile([128, 4096], F8, tag="w1", name="w1")
            bs_sb = wpool.tile([128, 12], F32, tag="bs", name="bs")
            w2_sb = wpool.tile([128, 1120], F8, tag="w2", name="w2")
            gm_sb = wpool.tile([128, 1024], F8, tag="gm", name="gm")
            ws_sb = wpool.tile([128, 512], BF16, tag="ws", name="ws")
            scr = wpool.tile([1, 1], F32, tag="scr", name="scr")
            nc.sync.dma_start(xt_sb[:, 0:512], xtp[:, 0:512])
            nc.sync.dma_start(xt_sb[:, 512:1024], xtp[:, 512:1024])
            nc.scalar.dma_start(w1_sb[:, 0:1024], w1p[:, 0:1024])
            nc.gpsimd.dma_start(w1_sb[:, 1024:2048], w1p[:, 1024:2048])
            nc.gpsimd.dma_start(w1_sb[:, 2048:3072], w1p[:, 2048:3072])
            nc.sync.dma_start(w1_sb[:, 3072:4096], w1p[:, 3072:4096])
            nc.scalar.dma_start(bs_sb[:], bsf[:])
            nc.gpsimd.dma_start(w2_sb[:], w2p[:])
            nc.sync.dma_start(gm_sb[:], gmat[:])
            nc.scalar.dma_start(ws_sb[:], wsb[:])
            # dummy Sigmoid: makes the act-table pass pick the set that
            # also holds Identity/Copy/Relu -> one table load total
            nc.scalar.activation(scr[:], bs_sb[0:1, 0:1], AF.Sigmoid)

            def w1ap(h, kp, m):
                t = ((m * 2 + h) * 4 + 2 * kp) * 128
                return _strided(w1_sb[:, t:t + 1], [[128, 2], [1, 128]])

            def w2ap(kp, m2):
                t = 2 * kp * 256 + m2 * 128
                return _strided(w2_sb[:, t:t + 1], [[256, 2], [1, 128]])

            # ---------------- layer 1: F1t/F2t = (32*W1h)^T @ X^T (fp8 DR) ----------------
            # Two rounds (m0,m1 then m2,m3) rotating through one 2-bank F
            # pool.  No SBUF drain: the pair-table adds read F straight from
            # PSUM; bias b1 and the /32 fold into the relu-cast (h8 kept x32).
            f_rounds = [None, None]

            def l1_round(g):
                fg = [fpool.tile([128, 2 * COLS], F32, tag=f"f{h}", name=f"f{g}{h}")
                      for h in range(2)]
                for mm in range(2):
                    m = 2 * g + mm
                    for kp in range(2):
                        for h in range(2):
                            nc.tensor.matmul(
                                fg[h][:, mm * COLS:(mm + 1) * COLS],
                                lhsT=w1ap(h, kp, m),
                                rhs=_strided(xt_sb[:, kp * 2 * COLS:kp * 2 * COLS + 1],
                                             [[COLS, 2], [1, COLS]]),
                                start=(kp == 0),
                                stop=(kp == 1),
                                perf_mode=DR,
                            )
                # PSUM-read DVE adds cost ~2x (access latency), and only one
                # input may be PSUM anyway: spill both halves to SBUF bf16
                # with plain copies (b1 + /32 live in the relu-cast).
                f0sb = fsb_pool.tile([128, 2 * COLS], BF16, tag="f0sb", name=f"f0sb{g}")
                f1sb = fsb_pool.tile([128, 2 * COLS], BF16, tag="f1sb", name=f"f1sb{g}")
                nc.scalar.activation(f0sb[:], fg[0][:], AF.Copy)
                if g == 0:
                    nc.vector.tensor_copy(f1sb[:], fg[1][:])
                else:
                    nc.scalar.activation(f1sb[:], fg[1][:], AF.Copy)
                f_rounds[g] = (f0sb, f1sb)

            # H8: fp8 relu'd pair table x32, col = modal*8192 + c*2048 + pair*8 + b
            h8 = hpool.tile([128, 16384], F8, tag="h8", name="h8")
            z_ps = []          # filled once the zps pool opens
            zs_t = [None, None]
            st_sb = [None, None]

            def h_chunk(modal, c, eng, nsplit=1):
                """Pair-table chunk c: H_bf = 32*(F1+F2) (DVE add from SBUF
                spills), then h8 = relu(hb + 32*b1) -> fp8 (kept x32).
                nsplit>1 pipelines the chunk in i-halves so downstream L2
                matmuls unblock earlier."""
                off = (c % 2) * COLS + modal * R
                fg = f_rounds[c // 2]
                hb = hbf_pool.tile([128, HC_M], BF16, tag="hb", name=f"hb{modal}{c}")
                bias = bs_sb[:, c:c + 1]
                ni = N // nsplit
                for half in range(nsplit):
                    in0 = _strided(fg[0][:, off + half * ni * BC:off + half * ni * BC + 1],
                                   [[BC, ni], [0, N], [1, BC]])
                    in1 = _strided(fg[1][:, off:off + 1], [[0, ni], [BC, N], [1, BC]])
                    outap = _strided(hb[:, half * ni * 128:half * ni * 128 + 1],
                                     [[N * BC, ni], [BC, N], [1, BC]])
                    nc.vector.tensor_tensor(out=outap, in0=in0, in1=in1, op=ALU.add)
                    base = modal * 8192 + c * 2048 + half * ni * 128
                    dst = h8[:, base:base + ni * 128]
                    hsl = hb[:, half * ni * 128:(half + 1) * ni * 128]
                    if eng == "act":
                        nc.scalar.activation(dst, hsl, AF.Relu, bias=bias)
                    else:
                        nc.vector.tensor_scalar(
                            out=dst, in0=hsl, scalar1=bias, scalar2=0.0,
                            op0=ALU.add, op1=ALU.max)

            def h2_drain_op(dst, ps, bias, eng):
                """psum -> h2 fp8: relu(ps + 128*b2) (h2 kept x128)."""
                if eng == "act":
                    nc.scalar.activation(dst, ps[:], AF.Relu, bias=bias)
                else:
                    nc.vector.tensor_scalar(
                        out=dst, in0=ps[:], scalar1=bias, scalar2=0.0,
                        op0=ALU.add, op1=ALU.max)

            def l2l3(modal, drain_engs):
                """L2 (nn-outer, DR) + immediate h2 drain + DR-packed L3."""
                for nn in range(4):
                    nn_ps = []
                    for m2 in range(2):
                        ps = l2ps_pool.tile([128, 512], F32, tag="l2", name="l2")
                        for kp in range(2):
                            nc.tensor.matmul(
                                ps[:],
                                lhsT=w2ap(kp, m2),
                                rhs=_strided(
                                    h8[:, modal * 8192 + kp * 4096 + nn * 512:
                                       modal * 8192 + kp * 4096 + nn * 512 + 1],
                                    [[2048, 2], [1, 512]]),
                                start=(kp == 0),
                                stop=(kp == 1),
                                perf_mode=DR,
                            )
                        nn_ps.append(ps)
                    h2 = h2pool.tile([128, 1024], F8, tag="h2", name="h2")
                    for m2 in range(2):
                        h2_drain_op(h2[:, m2 * 512:(m2 + 1) * 512], nn_ps[m2],
                                    bs_sb[:, 4 + m2:5 + m2], drain_engs[nn][m2])
                    # L3: z rows 32*nn (K=256 over two m2 accumulations)
                    cp = 32 * nn
                    for m2 in range(2):
                        nc.tensor.matmul(
                            z_ps[modal][cp:cp + 32, :],
                            lhsT=w2_sb[:, 1024 + m2 * 32:1024 + (m2 + 1) * 32],
                            rhs=h2[:, m2 * 512:(m2 + 1) * 512],
                            start=(m2 == 0),
                            stop=(m2 == 1),
                            tile_position=(0, cp),
                        )
                # fused sigmoid drain: zs = sigmoid(z/128 + b3), bf16
                zs = stpool.tile([128, 512], BF16, tag=f"zs{modal}", name=f"zs{modal}")
                nc.scalar.activation(
                    zs[:], z_ps[modal][:], AF.Sigmoid,
                    bias=float(b3), scale=1.0 / (H2S * WS3),
                )
                zs_t[modal] = zs

            def zeb_pivot(modal, qengs):
                """zs -> zeb [e%128, ec*8+b] via 4 strided DMAs."""
                zs = zs_t[modal]
                zeb = stpool.tile([128, 16], BF16, tag=f"zeb{modal}", name=f"zeb{modal}")
                pstep = zs[:].ap[0][0]
                for ec in range(2):
                    for g2 in range(2):
                        src = bass.AP(
                            zs[:].tensor, (64 * ec + 32 * g2) * pstep,
                            [[32 * pstep, 1], [8, 64], [1, 8]],
                        )
                        qengs[2 * ec + g2].dma_start(
                            zeb[64 * g2:64 * g2 + 64, ec * 8:(ec + 1) * 8], src,
                            single_packet=True)
                return zeb

            def gather_mm(modal, zeb, qengs, cp_eng="dve"):
                """one-hot gather (DR-packed over ec) + pivot to S_T [l, (p,b)]."""
                s1 = stpool.tile([128, 32], BF16, tag=f"s1{modal}", name=f"s1{modal}")
                for plc in range(4):
                    s1t = l2ps_pool.tile([128, 512], F32, tag="l2", name="s1ps")
                    s1ps = s1t[:, 0:8]
                    for ec in range(2):
                        nc.tensor.matmul(
                            s1ps,
                            lhsT=gm_sb[:, ec * 512 + plc * 128:ec * 512 + (plc + 1) * 128],
                            rhs=zeb[:, ec * 8:(ec + 1) * 8],
                            start=(ec == 0),
                            stop=(ec == 1),
                        )
                    if cp_eng == "act":
                        nc.scalar.activation(s1[:, 8 * plc:8 * plc + 8], s1ps, AF.Copy)
                    else:
                        nc.vector.tensor_copy(s1[:, 8 * plc:8 * plc + 8], s1ps)
                st = stpool.tile([L, P * BC], BF16, tag=f"st{modal}", name=f"st{modal}")
                for c in range(4):
                    qengs[c].dma_start(st[2 * c:2 * c + 2, :], s1[:, 8 * c:8 * c + 8],
                                       single_packet=True)
                st_sb[modal] = st

            def stage2(modal, at_eng):
                at_sb = ypool.tile([128, 4 * 512], F8, tag=f"at{modal}", name=f"at{modal}")
                aps_t = []
                for fc in range(4):
                    ap_ps = l2ps_pool.tile([128, 512], F32, tag="l2", name="a")
                    nc.tensor.matmul(
                        ap_ps[:],
                        lhsT=ws_sb[0:L, fc * 128:(fc + 1) * 128],
                        rhs=st_sb[modal][:],
                        start=True,
                        stop=True,
                    )
                    aps_t.append(ap_ps)
                for i, fc in enumerate((0, 2, 1, 3)):
                    # at = relu(16*(a + b4)) -> fp8 (W4/b4 pre-scaled x16);
                    # y-pair order (0,2),(1,3), engines alternating so each
                    # DR y matmul's pair of chunks drains in parallel
                    dst = at_sb[:, fc * 512:(fc + 1) * 512]
                    bias = bs_sb[:, 6 + fc:7 + fc]
                    eng = at_eng if i % 2 == 0 else ("dve" if at_eng == "act" else "act")
                    if eng == "act":
                        nc.scalar.activation(dst, aps_t[fc][:], AF.Relu, bias=bias)
                    else:
                        nc.vector.tensor_scalar(
                            out=dst, in0=aps_t[fc][:], scalar1=bias, scalar2=0.0,
                            op0=ALU.add, op1=ALU.max,
                        )
                y_pt = l2ps_pool.tile([128, 512], F32, tag="l2", name="yps")
                for fc in range(2):
                    # DR over d-chunk pairs (fc, fc+2); M=2 duplicated rows
                    # (dual-fp8 ldweights rejects M=1)
                    nc.tensor.matmul(
                        y_pt[0:2, :],
                        lhsT=_strided(w2_sb[:, 1088 + 2 * fc:1089 + 2 * fc], [[16, 2], [1, 2]]),
                        rhs=_strided(at_sb[:, fc * 512:fc * 512 + 1], [[1024, 2], [1, 512]]),
                        start=(fc == 0),
                        stop=(fc == 1),
                        perf_mode=DR,
                    )
                y = ypool.tile([1, 512], F32, tag=f"y{modal}", name=f"y{modal}")
                nc.vector.tensor_copy(y[:], y_pt[0:1, :])
                nc.sync.dma_start(out[modal:modal + 1, :], y[:])

            # ---------------- schedule (DAG; tile scheduler interleaves) ----------------
            # GpSimd tensor ops are ~17x slower than ACT/DVE (software DSP) --
            # elementwise stays on ACT/DVE only; GpSimd handles DMA triggers.
            # Per-engine queues execute IN ORDER: emission order below is
            # hand-scheduled so no queue head stalls on a long-latency dep
            # while ready work sits behind it.
            cast_eng = {(0, 0): "dve", (0, 1): "act", (0, 2): "dve", (0, 3): "act",
                        (1, 0): "act", (1, 1): "dve", (1, 2): "dve", (1, 3): "dve"}
            z_ps.append(zps_pool.tile([128, 512], F32, tag="z0", name="z0"))
            z_ps.append(zps_pool.tile([128, 512], F32, tag="z1", name="z1"))
            l1_round(0)
            for c in range(2):
                h_chunk(0, c, cast_eng[(0, c)])
            # round 1 rotates into the same F banks (WAR on the r0 spills)
            l1_round(1)
            for c in range(2, 4):
                h_chunk(0, c, cast_eng[(0, c)], nsplit=2)
            l2l3(0, drain_engs=[("act", "act")] * 4)
            for c in range(4):
                h_chunk(1, c, cast_eng[(1, c)], nsplit=(1 if c < 2 else 2))
            zeb0 = zeb_pivot(0, [nc.sync, nc.gpsimd, nc.scalar, nc.gpsimd])
            l2l3(1, drain_engs=[("act", "dve")] * 4)
            gather_mm(0, zeb0, [nc.sync, nc.scalar, nc.sync, nc.scalar])
            zeb1 = zeb_pivot(1, [nc.gpsimd, nc.gpsimd, nc.sync, nc.scalar])
            gather_mm(1, zeb1, [nc.gpsimd, nc.sync, nc.scalar, nc.gpsimd], cp_eng="act")
            stage2(0, "dve")
            stage2(1, "act")
            psum_stack.close()

    nc.compile()
    return nc


_PROG_CACHE: dict = {}


def _get_program(b3: float, b5: float):
    key = (round(float(b3), 12), round(float(b5), 12))
    if key not in _PROG_CACHE:
        prev = tile.TileContext._drain_and_barrier
        tile.TileContext._drain_and_barrier = _lean_drain_and_barrier
        try:
            _PROG_CACHE[key] = _build_program(b3, b5)
        finally:
            tile.TileContext._drain_and_barrier = prev
    return _PROG_CACHE[key]


def _prep_inputs(inputs):
    """Host-side restructuring. Returns per-core input maps."""
    img = np.asarray(inputs["img_features"], np.float32)
    txt = np.asarray(inputs["text_features"], np.float32)
    paths = np.asarray(inputs["paths"])
    W1 = np.asarray(inputs["W1"], np.float32)
    W2 = np.asarray(inputs["W2"], np.float32)
    W3 = np.asarray(inputs["W3"], np.float32)
    W4 = np.asarray(inputs["W4"], np.float32)
    W5 = np.asarray(inputs["W5"], np.float32)
    b1 = np.asarray(inputs["b1"], np.float32)
    b2 = np.asarray(inputs["b2"], np.float32)
    b4 = np.asarray(inputs["b4"], np.float32)

    # w1p[r, ((m*2+h)*4+k)*128 + c] = 32*W1[(h*4+k)*128 + r, m*128 + c]
    w1p = np.ascontiguousarray(
        (WS1 * W1).reshape(2, 4, 128, 4, 128).transpose(2, 3, 0, 1, 4)
        .reshape(128, 4096).astype(F8NP)
    )
    w2p = np.zeros((128, 1120), F8NP)
    w2p[:, 0:1024] = (
        (WS2 * W2).reshape(4, 128, D2).transpose(1, 0, 2).reshape(128, 1024).astype(F8NP)
    )
    w3col = (WS3 * W3[:, 0]).reshape(2, 128).T  # [128, 2]
    w2p[:, 1024:1088] = np.repeat(w3col[:, :, None], 32, axis=2).reshape(128, 64).astype(F8NP)
    w5r = (WS5 * W5[:, 0]).reshape(4, 128).T.astype(F8NP)  # [128, d-chunk]
    for fc in range(2):
        for t in range(2):
            for m in range(2):
                # k-tile stride 16 cols (16B) for dual-fp8 ldweights alignment
                w2p[:, 1088 + 16 * t + 2 * fc + m] = w5r[:, fc + 2 * t]
    wsb = np.zeros((128, 512), BF16NP)
    wsb[0:L, 0:512] = AS * W4[:L]
    bsf = np.zeros((128, 12), np.float32)
    bsf[:, 0:4] = WS1 * b1.reshape(4, 128).T
    bsf[:, 4:6] = H2S * b2.reshape(2, 128).T  # 128*b2
    bsf[:, 6:10] = AS * b4.reshape(4, 128).T

    e = (paths[:, :-1].astype(np.int64) * N + paths[:, 1:].astype(np.int64))  # [P, L]
    e_flat = e.T.reshape(-1)  # index (l*64+p)
    G = np.zeros((NPAIR, P * L), np.float32)  # [256, 512]
    G[e_flat, np.arange(P * L)] = 1.0
    gmat = np.ascontiguousarray(
        G.reshape(2, 128, 4, 128).transpose(1, 0, 2, 3).reshape(128, 1024).astype(F8NP)
    )

    shared = dict(w1p=w1p, w2p=w2p, wsb=wsb, bsf=bsf, gmat=gmat)
    in_maps = []
    for c in range(NCORES):
        bs = slice(c * BC, (c + 1) * BC)
        xi = img[bs].transpose(2, 1, 0).reshape(D, R)   # [512, (i,b)]
        xx = txt[bs].transpose(2, 1, 0).reshape(D, R)
        xt2 = np.concatenate([xi, xx], axis=1)           # [512, 256]
        xtp = np.ascontiguousarray(
            xt2.reshape(4, 128, 256).transpose(1, 0, 2).reshape(128, 1024).astype(F8NP)
        )
        in_maps.append(dict(shared, xtp=xtp))
    return in_maps


def _ensure_ntff_hook():
    """bass_utils expects antenv.axon_hooks for trace=True under axon; the
    installed antenv lacks it, but trn_agent_boot has the ctypes impl."""
    import types

    if "antenv.axon_hooks" in sys.modules:
        return
    try:
        import trn_agent_boot.trn_boot as tb

        hook = tb._ntff_profile_via_ctypes("/opt/axon/libaxon_pjrt.so")
    except Exception:
        hook = None
    mod = types.ModuleType("antenv.axon_hooks")
    mod.get_axon_ntff_profile_hook = lambda: hook
    mod.set_axon_ntff_profile_hook = lambda h: None
    sys.modules["antenv.axon_hooks"] = mod


def _run(inputs, trace=False):
    b3 = float(np.asarray(inputs["b3"]).reshape(-1)[0])
    b5 = float(np.asarray(inputs["b5"]).reshape(-1)[0])
    nc = _get_program(b3, b5)
    in_maps = _prep_inputs(inputs)
    if trace:
        _ensure_ntff_hook()
    res = run_bass_kernel_spmd(nc, in_maps, core_ids=list(range(NCORES)), trace=trace)
    outs = []
    for c in range(NCORES):
        ypre = res.results[c]["out"].astype(np.float64) / YS  # [2, 512] cols (p,b)
        y = 1.0 / (1.0 + np.exp(-(ypre + b5)))
        m = (y[0] * y[1]).reshape(P, BC)
        outs.append(np.sqrt(m.max(axis=0)))
    full = np.concatenate(outs).reshape(B, 1).astype(np.float32)
    return full, res


def kernel(**inputs) -> np.ndarray:
    full, _ = _run(inputs)
    return full


def kernel_with_stats(**inputs):
    full, res = _run(inputs, trace=True)
    return full, res


# revision 55
# speedup vs baseline: 1.1904x; 1.1904x over previous
"""Trainium2 Bass kernel for nn_CausalMatchingModule (fp8 DoubleRow, v2).

Reference computation (B=64, N=16, D=512, P=64, L=8, D2=256):
  per modality feats in {img, text}:
    src = feats[:, paths[:, :-1]]          # [B,P,L,D]
    dst = feats[:, paths[:, 1:]]           # [B,P,L,D]
    h  = relu(concat(src,dst) @ W1 + b1)   # [B,P,L,D]
    h  = relu(h @ W2 + b2)                 # [B,P,L,D2]
    s  = sigmoid(h @ W3 + b3)[...,0]       # [B,P,L]
    pf = pad(s, L->D)                      # only first L rows of W4 matter
    a  = relu(pf @ W4 + b4)                # [B,P,D]
    y  = sigmoid(a @ W5 + b5)              # [B,P,1]
  out = max_p sqrt(y_img * y_text)         # [B,1]

Key algebraic restructuring (same as baseline):
  * concat(src,dst)@W1 factors through the gather: per-node projections
    F1/F2 on B*N rows; pair (i,j) pre-act = F1[b,i] + F2[b,j] + b1.
  * paths reference only N*N = 256 distinct pairs, so the edge MLP runs
    on the 256-pair table per batch; per-edge scores gathered afterwards
    with a one-hot matmul + strided pivot DMAs.
  * pf@W4 = s @ W4[:L,:];  max_p sqrt(m) = sqrt(max_p m).

Optimizations over the 62.9us baseline (now ~55us):
  * dummy Sigmoid first on ACT pins the sigmoid-bearing act-table set ->
    no 1.3us mid-kernel table reload before the z drain.
  * lean TileContext teardown: single sync drain + gpsimd sem clears
    replace the stock pool-release barriers (the NEFF-level event-sem
    barrier ring at program end is runtime-fixed and remains).
  * L1 in two rounds rotating 2 PSUM banks; F spilled to SBUF with
    plain copies (b1 and the /32 fold into the pair-table relu-cast,
    h8 kept x32, h2 x128) -- no biased Identity/Copy drains.
  * elementwise strictly on ACT+DVE (GpSimd tensor ops are ~17x slower
    software loops; GpSimd only triggers DMAs), hand-balanced: adds +
    half the casts on DVE, rest on ACT; h2 drains split per-m2 across
    both engines for modality 1; c2/c3 chunks built in i-halves so L2
    kp1 unblocks earlier.
  * final y matmul fp8 DoubleRow (W4/b4 scaled x16 so `at` drains to
    fp8, W5 x32 fp8 with 16B-aligned dual-row packing); host /512.
    `at` drains alternate engines in y-pair order (0,2),(1,3).
  * endgame: modality-1 legs (zeb pivot, gather, st pivot) emitted
    ahead of stage2(0) so the last chain isn't queued behind slack
    work; pivot DMA triggers spread over SP/ACT/GpSimd.
  * input DMA triggers spread across engines, first-needed first;
    xt split so the first L1 matmul starts ~4.5us in.

Sharding: data-parallel over batch. Core c handles batches [8c, 8c+8);
weights replicated. Both modalities processed by one program.
"""

import os
import sys

import ml_dtypes
import numpy as np

BF16NP = ml_dtypes.bfloat16
F8NP = ml_dtypes.float8_e4m3

for _p in ("/opt/trn_rl_repo",):
    if os.path.isdir(_p) and _p not in sys.path:
        sys.path.append(_p)

import concourse.bacc as bacc
import concourse.bass as bass
import concourse.tile as tile
from concourse import mybir
from concourse.bass_utils import run_bass_kernel_spmd
from concourse.vector_clock import ScopedClock

F32 = mybir.dt.float32
BF16 = mybir.dt.bfloat16
F8 = mybir.dt.float8e4
AF = mybir.ActivationFunctionType
ALU = mybir.AluOpType
DR = mybir.MatmulPerfMode.DoubleRow

B, N, D, P, L = 64, 16, 512, 64, 8
D2 = D // 2
NCORES = 8
BC = B // NCORES          # 8 batches per core
R = BC * N                # 128 rows per modality
COLS = 2 * R              # 256 layer-1 rhs columns (both modalities)
NPAIR = N * N             # 256 (i,j) pairs per batch
HC_M = BC * NPAIR         # 2048 H columns per modality
WS1 = 32.0                # host scale on W1 (fp8 range)
WS2 = 4.0                 # host scale on W2
WS3 = 32.0                # host scale on W3
H2S = 128.0               # h2 stored x128 (h8 kept x32, W2 x4)
AS = 16.0                 # host scale on W4/b4 -> `at` stored x16 in fp8
WS5 = 32.0                # host scale on W5 (fp8 range)
YS = AS * WS5             # y psum = 512 * ypre


def _lean_drain_and_barrier(self, tick_clock, wait_clock):
    """Teardown: single sync drain + gpsimd sem clears.

    Replaces the stock two event-semaphore all-engine barrier rings
    (~6us of serial event-queue latency).  Correctness: the sync drain
    waits for every semaphore's final value (all instructions + DMAs
    complete); gpsimd then clears the semaphores so the NEFF stays
    re-runnable; every other engine simply halts after its last real
    instruction.
    """
    nc = self.nc
    drain_inst = nc.sync.drain()
    wait_clock.add_sem_waits(drain_inst.ins, ScopedClock({None: tick_clock.global_clock}))
    sem = nc.alloc_semaphore("lean_teardown")
    drain_inst.then_inc(sem, 1)
    nc.gpsimd.wait_ge(sem, 1)
    popped = nc._tile_sem_poison_stack.pop()
    assert popped is self._sem_poison
    nc.clear_and_free_semaphores(list(self.sems.allocated().values()))
    nc.gpsimd.sem_clear(range(sem.num, sem.num + 1))


def _strided(base: bass.AP, dims) -> bass.AP:
    """AP with explicit free [step,count] dims (incl. stride-0 broadcast)."""
    return bass.AP(base.tensor, base.offset, [list(base.ap[0])] + [list(d) for d in dims])


def _build_program(b3: float, b5: float):
    nc = bacc.Bacc("TRN2", target_bir_lowering=False)

    # Consolidated inputs:
    #  xtp  [128, 1024] fp8: col = k*256 + modal*128 + i*8 + b   (X^T k-chunks)
    #  w1p  [128, 4096] fp8: col = ((m*2+h)*4+k)*128 + c         (32*W1)
    #  w2p  [128, 1120] fp8: [0:1024] 4*W2 (k*256+m2*128+c) | [1024:1088] 32*W3
    #                        rep x32 | [1088:1120] 32*W5 dual-row packed
    #  wsb  [128, 512]  bf16: 16*W4' (only rows 0:L nonzero)
    #  gmat [128, 1024] fp8: one-hot gather, col = ec*512 + plc*128 + r
    #  bsf  [128, 12]   f32: [0:4] 32*b1 | [4:6] 128*b2 | [6:10] 16*b4
    xtp = nc.dram_tensor("xtp", [128, 1024], F8, kind="ExternalInput")
    w1p = nc.dram_tensor("w1p", [128, 4096], F8, kind="ExternalInput")
    w2p = nc.dram_tensor("w2p", [128, 1120], F8, kind="ExternalInput")
    wsb = nc.dram_tensor("wsb", [128, 512], BF16, kind="ExternalInput")
    gmat = nc.dram_tensor("gmat", [128, 1024], F8, kind="ExternalInput")
    bsf = nc.dram_tensor("bsf", [128, 12], F32, kind="ExternalInput")
    out = nc.dram_tensor("out", [2, 512], F32, kind="ExternalOutput")

    with tile.TileContext(nc) as tc:
        from contextlib import ExitStack
        with (
            tc.tile_pool(name="wpool", bufs=1) as wpool,
            tc.tile_pool(name="fsb", bufs=2) as fsb_pool,
            tc.tile_pool(name="hbf", bufs=4) as hbf_pool,
            tc.tile_pool(name="hpool", bufs=1) as hpool,
            tc.tile_pool(name="h2pool", bufs=5) as h2pool,
            tc.tile_pool(name="stpool", bufs=1) as stpool,
            tc.tile_pool(name="ypool", bufs=1) as ypool,
        ):
            psum_stack = ExitStack()
            fpool = psum_stack.enter_context(tc.tile_pool(name="fps", bufs=1, space="PSUM"))
            l2ps_pool = psum_stack.enter_context(tc.tile_pool(name="l2ps", bufs=4, space="PSUM"))
            zps_pool = psum_stack.enter_context(tc.tile_pool(name="zps", bufs=1, space="PSUM"))
            # ---------------- input loads (first-needed first) ----------------
            xt_sb = wpool.tile([128, 1024], F8, tag="xt", name="xt")
            w1_sb = wpool.tile([128, 4096], F8, tag="w1", name="w1")
            bs_sb = wpool.tile([128, 12], F32, tag="bs", name="bs")
            w2_sb = wpool.tile([128, 1120], F8, tag="w2", name="w2")
            gm_sb = wpool.tile([128, 1024], F8, tag="gm", name="gm")
            ws_sb = wpool.tile([128, 512], BF16, tag="ws", name="ws")
            scr = wpool.tile([1, 1], F32, tag="scr", name="scr")
            nc.sync.dma_start(xt_sb[:, 0:512], xtp[:, 0:512])
            nc.sync.dma_start(xt_sb[:, 512:1024], xtp[:, 512:1024])
            nc.scalar.dma_start(w1_sb[:, 0:1024], w1p[:, 0:1024])
            nc.gpsimd.dma_start(w1_sb[:, 1024:2048], w1p[:, 1024:2048])
            nc.gpsimd.dma_start(w1_sb[:, 2048:3072], w1p[:, 2048:3072])
            nc.sync.dma_start(w1_sb[:, 3072:4096], w1p[:, 3072:4096])
            nc.scalar.dma_start(bs_sb[:], bsf[:])
            nc.gpsimd.dma_start(w2_sb[:], w2p[:])
            nc.sync.dma_start(gm_sb[:], gmat[:])
            nc.scalar.dma_start(ws_sb[:], wsb[:])
            # dummy Sigmoid: makes the act-table pass pick the set that
            # also holds Identity/Copy/Relu -> one table load total
            nc.scalar.activation(scr[:], bs_sb[0:1, 0:1], AF.Sigmoid)

            def w1ap(h, kp, m):
                t = ((m * 2 + h) * 4 + 2 * kp) * 128
                return _strided(w1_sb[:, t:t + 1], [[128, 2], [1, 128]])

            def w2ap(kp, m2):
                t = 2 * kp * 256 + m2 * 128
                return _strided(w2_sb[:, t:t + 1], [[256, 2], [1, 128]])

            # ---------------- layer 1: F1t/F2t = (32*W1h)^T @ X^T (fp8 DR) ----------------
            # Two rounds (m0,m1 then m2,m3) rotating through one 2-bank F
            # pool.  No SBUF drain: the pair-table adds read F straight from
            # PSUM; bias b1 and the /32 fold into the relu-cast (h8 kept x32).
            f_rounds = [None, None]

            def l1_round(g):
                fg = [fpool.tile([128, 2 * COLS], F32, tag=f"f{h}", name=f"f{g}{h}")
                      for h in range(2)]
                for mm in range(2):
                    m = 2 * g + mm
                    for kp in range(2):
                        for h in range(2):
                            nc.tensor.matmul(
                                fg[h][:, mm * COLS:(mm + 1) * COLS],
                                lhsT=w1ap(h, kp, m),
                                rhs=_strided(xt_sb[:, kp * 2 * COLS:kp * 2 * COLS + 1],
                                             [[COLS, 2], [1, COLS]]),
                                start=(kp == 0),
                                stop=(kp == 1),
                                perf_mode=DR,
                            )
                # PSUM-read DVE adds cost ~2x (access latency), and only one
                # input may be PSUM anyway: spill both halves to SBUF bf16
                # with plain copies (b1 + /32 live in the relu-cast).
                f0sb = fsb_pool.tile([128, 2 * COLS], BF16, tag="f0sb", name=f"f0sb{g}")
                f1sb = fsb_pool.tile([128, 2 * COLS], BF16, tag="f1sb", name=f"f1sb{g}")
                nc.scalar.activation(f0sb[:], fg[0][:], AF.Copy)
                if g == 0:
                    nc.vector.tensor_copy(f1sb[:], fg[1][:])
                else:
                    nc.scalar.activation(f1sb[:], fg[1][:], AF.Copy)
                f_rounds[g] = (f0sb, f1sb)

            # H8: fp8 relu'd pair table x32, col = modal*8192 + c*2048 + pair*8 + b
            h8 = hpool.tile([128, 16384], F8, tag="h8", name="h8")
            z_ps = []          # filled once the zps pool opens
            zs_t = [None, None]
            st_sb = [None, None]

            def h_chunk(modal, c, eng, nsplit=1):
                """Pair-table chunk c: H_bf = 32*(F1+F2) (DVE add from SBUF
                spills), then h8 = relu(hb + 32*b1) -> fp8 (kept x32).
                nsplit>1 pipelines the chunk in i-halves so downstream L2
                matmuls unblock earlier."""
                off = (c % 2) * COLS + modal * R
                fg = f_rounds[c // 2]
                hb = hbf_pool.tile([128, HC_M], BF16, tag="hb", name=f"hb{modal}{c}")
                bias = bs_sb[:, c:c + 1]
                ni = N // nsplit
                for half in range(nsplit):
                    in0 = _strided(fg[0][:, off + half * ni * BC:off + half * ni * BC + 1],
                                   [[BC, ni], [0, N], [1, BC]])
                    in1 = _strided(fg[1][:, off:off + 1], [[0, ni], [BC, N], [1, BC]])
                    outap = _strided(hb[:, half * ni * 128:half * ni * 128 + 1],
                                     [[N * BC, ni], [BC, N], [1, BC]])
                    nc.vector.tensor_tensor(out=outap, in0=in0, in1=in1, op=ALU.add)
                    base = modal * 8192 + c * 2048 + half * ni * 128
                    dst = h8[:, base:base + ni * 128]
                    hsl = hb[:, half * ni * 128:(half + 1) * ni * 128]
                    if eng == "act":
                        nc.scalar.activation(dst, hsl, AF.Relu, bias=bias)
                    else:
                        nc.vector.tensor_scalar(
                            out=dst, in0=hsl, scalar1=bias, scalar2=0.0,
                            op0=ALU.add, op1=ALU.max)

            def h2_drain_op(dst, ps, bias, eng):
                """psum -> h2 fp8: relu(ps + 128*b2) (h2 kept x128)."""
                if eng == "act":
                    nc.scalar.activation(dst, ps[:], AF.Relu, bias=bias)
                else:
                    nc.vector.tensor_scalar(
                        out=dst, in0=ps[:], scalar1=bias, scalar2=0.0,
                        op0=ALU.add, op1=ALU.max)

            def l2l3(modal, drain_engs):
                """L2 (nn-outer, DR) + immediate h2 drain + DR-packed L3."""
                for nn in range(4):
                    nn_ps = []
                    for m2 in range(2):
                        ps = l2ps_pool.tile([128, 512], F32, tag="l2", name="l2")
                        for kp in range(2):
                            nc.tensor.matmul(
                                ps[:],
                                lhsT=w2ap(kp, m2),
                                rhs=_strided(
                                    h8[:, modal * 8192 + kp * 4096 + nn * 512:
                                       modal * 8192 + kp * 4096 + nn * 512 + 1],
                                    [[2048, 2], [1, 512]]),
                                start=(kp == 0),
                                stop=(kp == 1),
                                perf_mode=DR,
                            )
                        nn_ps.append(ps)
                    h2 = h2pool.tile([128, 1024], F8, tag="h2", name="h2")
                    for m2 in range(2):
                        h2_drain_op(h2[:, m2 * 512:(m2 + 1) * 512], nn_ps[m2],
                                    bs_sb[:, 4 + m2:5 + m2], drain_engs[nn][m2])
                    # L3: z rows 32*nn (K=256 over two m2 accumulations)
                    cp = 32 * nn
                    for m2 in range(2):
                        nc.tensor.matmul(
                            z_ps[modal][cp:cp + 32, :],
                            lhsT=w2_sb[:, 1024 + m2 * 32:1024 + (m2 + 1) * 32],
                            rhs=h2[:, m2 * 512:(m2 + 1) * 512],
                            start=(m2 == 0),
                            stop=(m2 == 1),
                            tile_position=(0, cp),
                        )
                # fused sigmoid drain: zs = sigmoid(z/128 + b3), bf16
                zs = stpool.tile([128, 512], BF16, tag=f"zs{modal}", name=f"zs{modal}")
                nc.scalar.activation(
                    zs[:], z_ps[modal][:], AF.Sigmoid,
                    bias=float(b3), scale=1.0 / (H2S * WS3),
                )
                zs_t[modal] = zs

            def zeb_pivot(modal, qengs):
                """zs -> zeb [e%128, ec*8+b] via 4 strided DMAs."""
                zs = zs_t[modal]
                zeb = stpool.tile([128, 16], BF16, tag=f"zeb{modal}", name=f"zeb{modal}")
                pstep = zs[:].ap[0][0]
                for ec in range(2):
                    for g2 in range(2):
                        src = bass.AP(
                            zs[:].tensor, (64 * ec + 32 * g2) * pstep,
                            [[32 * pstep, 1], [8, 64], [1, 8]],
                        )
                        qengs[2 * ec + g2].dma_start(
                            zeb[64 * g2:64 * g2 + 64, ec * 8:(ec + 1) * 8], src,
                            single_packet=True)
                return zeb

            def gather_mm(modal, zeb, qengs, cp_eng="dve"):
                """one-hot gather (DR-packed over ec) + pivot to S_T [l, (p,b)]."""
                s1 = stpool.tile([128, 32], BF16, tag=f"s1{modal}", name=f"s1{modal}")
                for plc in range(4):
                    s1t = l2ps_pool.tile([128, 512], F32, tag="l2", name="s1ps")
                    s1ps = s1t[:, 0:8]
                    for ec in range(2):
                        nc.tensor.matmul(
                            s1ps,
                            lhsT=gm_sb[:, ec * 512 + plc * 128:ec * 512 + (plc + 1) * 128],
                            rhs=zeb[:, ec * 8:(ec + 1) * 8],
                            start=(ec == 0),
                            stop=(ec == 1),
                        )
                    if cp_eng == "act":
                        nc.scalar.activation(s1[:, 8 * plc:8 * plc + 8], s1ps, AF.Copy)
                    else:
                        nc.vector.tensor_copy(s1[:, 8 * plc:8 * plc + 8], s1ps)
                st = stpool.tile([L, P * BC], BF16, tag=f"st{modal}", name=f"st{modal}")
                for c in range(4):
                    qengs[c].dma_start(st[2 * c:2 * c + 2, :], s1[:, 8 * c:8 * c + 8],
                                       single_packet=True)
                st_sb[modal] = st

            def stage2(modal, at_eng):
                at_sb = ypool.tile([128, 4 * 512], F8, tag=f"at{modal}", name=f"at{modal}")
                aps_t = []
                for fc in range(4):
                    ap_ps = l2ps_pool.tile([128, 512], F32, tag="l2", name="a")
                    nc.tensor.matmul(
                        ap_ps[:],
                        lhsT=ws_sb[0:L, fc * 128:(fc + 1) * 128],
                        rhs=st_sb[modal][:],
                        start=True,
                        stop=True,
                    )
                    aps_t.append(ap_ps)
                for i, fc in enumerate((0, 2, 1, 3)):
                    # at = relu(16*(a + b4)) -> fp8 (W4/b4 pre-scaled x16);
                    # y-pair order (0,2),(1,3), engines alternating so each
                    # DR y matmul's pair of chunks drains in parallel
                    dst = at_sb[:, fc * 512:(fc + 1) * 512]
                    bias = bs_sb[:, 6 + fc:7 + fc]
                    eng = at_eng if i % 2 == 0 else ("dve" if at_eng == "act" else "act")
                    if eng == "act":
                        nc.scalar.activation(dst, aps_t[fc][:], AF.Relu, bias=bias)
                    else:
                        nc.vector.tensor_scalar(
                            out=dst, in0=aps_t[fc][:], scalar1=bias, scalar2=0.0,
                            op0=ALU.add, op1=ALU.max,
                        )
                y_pt = l2ps_pool.tile([128, 512], F32, tag="l2", name="yps")
                for fc in range(2):
                    # DR over d-chunk pairs (fc, fc+2); M=2 duplicated rows
                    # (dual-fp8 ldweights rejects M=1)
                    nc.tensor.matmul(
                        y_pt[0:2, :],
                        lhsT=_strided(w2_sb[:, 1088 + 2 * fc:1089 + 2 * fc], [[16, 2], [1, 2]]),
                        rhs=_strided(at_sb[:, fc * 512:fc * 512 + 1], [[1024, 2], [1, 512]]),
                        start=(fc == 0),
                        stop=(fc == 1),
                        perf_mode=DR,
                    )
                y = ypool.tile([1, 512], F32, tag=f"y{modal}", name=f"y{modal}")
                nc.vector.tensor_copy(y[:], y_pt[0:1, :])
                nc.sync.dma_start(out[modal:modal + 1, :], y[:])

            # ---------------- schedule (DAG; tile scheduler interleaves) ----------------
            # GpSimd tensor ops are ~17x slower than ACT/DVE (software DSP) --
            # elementwise stays on ACT/DVE only; GpSimd handles DMA triggers.
            # Per-engine queues execute IN ORDER: emission order below is
            # hand-scheduled so no queue head stalls on a long-latency dep
            # while ready work sits behind it.
            cast_eng = {(0, 0): "dve", (0, 1): "act", (0, 2): "dve", (0, 3): "act",
                        (1, 0): "act", (1, 1): "dve", (1, 2): "dve", (1, 3): "dve"}
            z_ps.append(zps_pool.tile([128, 512], F32, tag="z0", name="z0"))
            z_ps.append(zps_pool.tile([128, 512], F32, tag="z1", name="z1"))
            l1_round(0)
            for c in range(2):
                h_chunk(0, c, cast_eng[(0, c)])
            # round 1 rotates into the same F banks (WAR on the r0 spills)
            l1_round(1)
            for c in range(2, 4):
                h_chunk(0, c, cast_eng[(0, c)], nsplit=2)
            l2l3(0, drain_engs=[("act", "act")] * 4)
            for c in range(4):
                h_chunk(1, c, cast_eng[(1, c)], nsplit=(1 if c < 2 else 2))
            zeb0 = zeb_pivot(0, [nc.sync, nc.gpsimd, nc.scalar, nc.gpsimd])
            l2l3(1, drain_engs=[("act", "dve")] * 4)
            gather_mm(0, zeb0, [nc.sync, nc.scalar, nc.sync, nc.scalar])
            zeb1 = zeb_pivot(1, [nc.gpsimd, nc.gpsimd, nc.sync, nc.scalar])
            gather_mm(1, zeb1, [nc.gpsimd, nc.sync, nc.scalar, nc.gpsimd], cp_eng="act")
            stage2(0, "dve")
            stage2(1, "act")
            psum_stack.close()

    nc.compile()
    return nc


_PROG_CACHE: dict = {}


def _get_program(b3: float, b5: float):
    key = (round(float(b3), 12), round(float(b5), 12))
    if key not in _PROG_CACHE:
        prev = tile.TileContext._drain_and_barrier
        tile.TileContext._drain_and_barrier = _lean_drain_and_barrier
        try:
            _PROG_CACHE[key] = _build_program(b3, b5)
        finally:
            tile.TileContext._drain_and_barrier = prev
    return _PROG_CACHE[key]


def _prep_inputs(inputs):
    """Host-side restructuring. Returns per-core input maps."""
    img = np.asarray(inputs["img_features"], np.float32)
    txt = np.asarray(inputs["text_features"], np.float32)
    paths = np.asarray(inputs["paths"])
    W1 = np.asarray(inputs["W1"], np.float32)
    W2 = np.asarray(inputs["W2"], np.float32)
    W3 = np.asarray(inputs["W3"], np.float32)
    W4 = np.asarray(inputs["W4"], np.float32)
    W5 = np.asarray(inputs["W5"], np.float32)
    b1 = np.asarray(inputs["b1"], np.float32)
    b2 = np.asarray(inputs["b2"], np.float32)
    b4 = np.asarray(inputs["b4"], np.float32)

    # w1p[r, ((m*2+h)*4+k)*128 + c] = 32*W1[(h*4+k)*128 + r, m*128 + c]
    w1p = np.ascontiguousarray(
        (WS1 * W1).reshape(2, 4, 128, 4, 128).transpose(2, 3, 0, 1, 4)
        .reshape(128, 4096).astype(F8NP)
    )
    w2p = np.zeros((128, 1120), F8NP)
    w2p[:, 0:1024] = (
        (WS2 * W2).reshape(4, 128, D2).transpose(1, 0, 2).reshape(128, 1024).astype(F8NP)
    )
    w3col = (WS3 * W3[:, 0]).reshape(2, 128).T  # [128, 2]
    w2p[:, 1024:1088] = np.repeat(w3col[:, :, None], 32, axis=2).reshape(128, 64).astype(F8NP)
    w5r = (WS5 * W5[:, 0]).reshape(4, 128).T.astype(F8NP)  # [128, d-chunk]
    for fc in range(2):
        for t in range(2):
            for m in range(2):
                # k-tile stride 16 cols (16B) for dual-fp8 ldweights alignment
                w2p[:, 1088 + 16 * t + 2 * fc + m] = w5r[:, fc + 2 * t]
    wsb = np.zeros((128, 512), BF16NP)
    wsb[0:L, 0:512] = AS * W4[:L]
    bsf = np.zeros((128, 12), np.float32)
    bsf[:, 0:4] = WS1 * b1.reshape(4, 128).T
    bsf[:, 4:6] = H2S * b2.reshape(2, 128).T  # 128*b2
    bsf[:, 6:10] = AS * b4.reshape(4, 128).T

    e = (paths[:, :-1].astype(np.int64) * N + paths[:, 1:].astype(np.int64))  # [P, L]
    e_flat = e.T.reshape(-1)  # index (l*64+p)
    G = np.zeros((NPAIR, P * L), np.float32)  # [256, 512]
    G[e_flat, np.arange(P * L)] = 1.0
    gmat = np.ascontiguousarray(
        G.reshape(2, 128, 4, 128).transpose(1, 0, 2, 3).reshape(128, 1024).astype(F8NP)
    )

    shared = dict(w1p=w1p, w2p=w2p, wsb=wsb, bsf=bsf, gmat=gmat)
    in_maps = []
    for c in range(NCORES):
        bs = slice(c * BC, (c + 1) * BC)
        xi = img[bs].transpose(2, 1, 0).reshape(D, R)   # [512, (i,b)]
        xx = txt[bs].transpose(2, 1, 0).reshape(D, R)
        xt2 = np.concatenate([xi, xx], axis=1)           # [512, 256]
        xtp = np.ascontiguousarray(
            xt2.reshape(4, 128, 256).transpose(1, 0, 2).reshape(128, 1024).astype(F8NP)
        )
        in_maps.append(dict(shared, xtp=xtp))
    return in_maps


def _ensure_ntff_hook():
    """bass_utils expects antenv.axon_hooks for trace=True under axon; the
    installed antenv lacks it, but trn_agent_boot has the ctypes impl."""
    import types

    if "antenv.axon_hooks" in sys.modules:
        return
    try:
        import trn_agent_boot.trn_boot as tb

        hook = tb._ntff_profile_via_ctypes("/opt/axon/libaxon_pjrt.so")
    except Exception:
        hook = None
    mod = types.ModuleType("antenv.axon_hooks")
    mod.get_axon_ntff_profile_hook = lambda: hook
    mod.set_axon_ntff_profile_hook = lambda h: None
    sys.modules["antenv.axon_hooks"] = mod


def _run(inputs, trace=False):
    b3 = float(np.asarray(inputs["b3"]).reshape(-1)[0])
    b5 = float(np.asarray(inputs["b5"]).reshape(-1)[0])
    nc = _get_program(b3, b5)
    in_maps = _prep_inputs(inputs)
    if trace:
        _ensure_ntff_hook()
    res = run_bass_kernel_spmd(nc, in_maps, core_ids=list(range(NCORES)), trace=trace)
    outs = []
    for c in range(NCORES):
        ypre = res.results[c]["out"].astype(np.float64) / YS  # [2, 512] cols (p,b)
        y = 1.0 / (1.0 + np.exp(-(ypre + b5)))
        m = (y[0] * y[1]).reshape(P, BC)
        outs.append(np.sqrt(m.max(axis=0)))
    full = np.concatenate(outs).reshape(B, 1).astype(np.float32)
    return full, res


def kernel(**inputs) -> np.ndarray:
    full, _ = _run(inputs)
    return full


def kernel_with_stats(**inputs):
    full, res = _run(inputs, trace=True)
    return full, res
